# revision 20
# baseline (speedup 1.0000x reference)
"""Trainium2 Bass kernel for nn_CrossWinAttention (windowed cross attention).

Contract: kernel(**inputs) takes FULL numpy inputs (as produced by
setup_inputs()) and returns the FULL output of the reference nn.Module.

Sharding: the (b, x*y) = 2*64 = 128 window cells are fully independent
(LN, QKV proj, per-cell attention, proj, mean over n are all cell-local).
16 cells per core on 8 NeuronCores; no collectives.

v2 design (software-pipelined, ACT-bound):
  Per cell i (tokens t = 64n + 8*w1 + w2, chunks c of 128):
    x      [128p, 3c, 128d]  f32 token-major (p = token % 128)
    LN stats on DVE (bn_stats/aggr, grouped rstd on ACT)
    normalize on Pool -> xn f16, one blocked DMA-transpose -> xT
    qT/kT = W^T xT (PE, psum ring) -> fp16 SBUF copies on Pool
    vf = xT_v chunks @ Wv -> [tok,hd] directly (no transpose), Pool copy
       into vf_aug [128,3,4,33] whose 33rd column is preset to 1.0
    scores^T per (kc, head-pair) into [128,2,512] PSUM (2-bank tiles,
       double buffered), exp on ACT -> E fp16 (6 exps/cell, the wall)
    A+l: out[q,33] = E_chunk^T @ [vf_h|1] accumulated over kc (36 matmuls
       of 33 cols -- l lands as column 32)
    rcp l on DVE (reciprocal_approx_fast), an = a * rcp (broadcast AP)
    an -> anT (DMA transpose), mean over n via strided DVE reduce -> am,
    z^T = Wp^T am (64 cols), DVE copy to zo, group DMA out.
  Engine streams are explicitly modulo-scheduled across cells so ACT
  (exp) never starves; PSUM is exactly 8 banks.
"""

import sys
import contextlib
import functools

sys.path.insert(0, "/opt/trn_rl_repo")

import numpy as np

import concourse.hw_specs as hw_specs

# --- ACT table-set pin ------------------------------------------------------
# bacc's insert_act_table_loads greedily picks the first table set containing
# each activation function; Ln and Exp then alternate between two sets and
# every cell pays ~2.7us table reloads.  Restrict Exp/Ln membership to the
# combined set (its index — and therefore the emitted act_func_set_id — is
# unchanged) so one load serves the whole kernel.
_orig_get_tables = hw_specs.get_activation_tables


@functools.cache
def _pinned_tables(arch):
    import concourse.mybir as mybir
    AF = mybir.ActivationFunctionType
    tabs = dict(_orig_get_tables(arch))
    out = {}
    for name, fns in tabs.items():
        if name != "natural_log_exp_and_others":
            fns = fns - {AF.Exp, AF.Ln}
        out[name] = fns
    return out


hw_specs.get_activation_tables = _pinned_tables

import concourse.bass as bass
import concourse.tile as tile
from concourse import bacc, mybir
from concourse.bass_utils import run_bass_kernel_spmd

# Problem shape (hardcoded per spec nn_CrossWinAttention_81346680586407)
B, N, X, Y, W1, W2, D = 2, 6, 8, 8, 8, 8, 128
H, DH = 4, 32
EPS = 1e-5
L = X * Y              # 64 cells per batch
Q = N * W1 * W2        # 384 tokens per cell
W = W1 * W2            # 64 output positions per cell
NCORES = 8
CELLS = (B * L) // NCORES   # 16 cells per core
GRP = 4                     # cells per input-DMA batch
F32 = mybir.dt.float32
F16 = mybir.dt.float16
AF = mybir.ActivationFunctionType
ALU = mybir.AluOpType
AXX = mybir.AxisListType.X


def _build(n_cells: int):
    """Build the per-core Bass graph (same program on all 8 cores)."""
    nc = bacc.Bacc("TRN2", target_bir_lowering=False, debug=False,
                   num_devices=NCORES)

    qkv_in = nc.dram_tensor("qkv_in", [n_cells, 3, N, W, D], F32,
                            kind="ExternalInput").ap()
    wq_d = nc.dram_tensor("wq", [D, H * DH], F16, kind="ExternalInput").ap()
    wk_d = nc.dram_tensor("wk", [D, H * DH], F16, kind="ExternalInput").ap()
    wv_d = nc.dram_tensor("wv", [D, H * DH], F16, kind="ExternalInput").ap()
    wp_d = nc.dram_tensor("wp", [H * DH, D], F16, kind="ExternalInput").ap()
    eps_d = nc.dram_tensor("epsc", [128, 1], F32, kind="ExternalInput").ap()
    out_d = nc.dram_tensor("out", [n_cells, D, W], F32, kind="ExternalOutput").ap()

    n_grp = (n_cells + GRP - 1) // GRP

    with tile.TileContext(nc) as tc, contextlib.ExitStack() as ctx:
        # ---- PSUM: exactly 8 banks -----------------------------------------
        # sc: 2-bank tiles x2 bufs = 4; P-ring [128,384] x3 = 3; azl 1.
        sc_ps = ctx.enter_context(tc.tile_pool(name="sc_ps", bufs=2, space="PSUM"))
        p_ps = ctx.enter_context(tc.tile_pool(name="p_ps", bufs=3, space="PSUM"))
        azl_ps = ctx.enter_context(tc.tile_pool(name="azl_ps", bufs=1, space="PSUM"))

        # ---- SBUF pools ----------------------------------------------------
        cpool = ctx.enter_context(tc.tile_pool(name="consts", bufs=1))
        xin = ctx.enter_context(tc.tile_pool(name="xin", bufs=8))
        stp = ctx.enter_context(tc.tile_pool(name="st", bufs=3))
        xnp = ctx.enter_context(tc.tile_pool(name="xn", bufs=3))
        xtp = ctx.enter_context(tc.tile_pool(name="xt", bufs=3))
        qkp = ctx.enter_context(tc.tile_pool(name="qk", bufs=3))
        vfp = ctx.enter_context(tc.tile_pool(name="vf", bufs=4))
        epool = ctx.enter_context(tc.tile_pool(name="E", bufs=3))
        rcpp = ctx.enter_context(tc.tile_pool(name="rcp", bufs=2))
        anp = ctx.enter_context(tc.tile_pool(name="an", bufs=2))
        antp = ctx.enter_context(tc.tile_pool(name="anT", bufs=3))
        amp = ctx.enter_context(tc.tile_pool(name="am", bufs=2))
        zop = ctx.enter_context(tc.tile_pool(name="zo", bufs=2))

        # ---- constants -----------------------------------------------------
        wq_sb = cpool.tile([D, H * DH], F16)
        wk_sb = cpool.tile([D, H * DH], F16)
        wv_sb = cpool.tile([D, H * DH], F16)
        wp_sb = cpool.tile([H * DH, D], F16)
        eps_sb = cpool.tile([128, 1], F32)
        for sb, dr in ((wq_sb, wq_d), (wk_sb, wk_d), (wv_sb, wv_d),
                       (wp_sb, wp_d), (eps_sb, eps_d)):
            nc.sync.dma_start(sb[:], dr[:])

        # vf_aug ring: preset the ones column once per buffer (never
        # overwritten afterwards -- the per-cell copy writes only [:, :, :, 0:32]).
        VF_BUFS = 4
        vfa_ring = []
        for _ in range(VF_BUFS):
            vfa0 = vfp.tile([128, 3, H, DH + 1], F16, tag="vfa")
            nc.gpsimd.memset(vfa0[:, :, :, DH:DH + 1], 1.0)
            vfa_ring.append(vfa0)

        # per-cell / per-group tile handles
        t_xin = {}   # i -> xin [128, 3t, 3c, D]
        t_st = {}    # g -> (st, mv, lnv, r9)
        t_xn = {}    # i -> xn
        t_xt = {}    # i -> xT
        t_qk = {}    # i -> (qT, kT)
        t_vfa = {}   # i -> vf_aug
        t_E = {}     # i -> E
        t_P = {}     # ('pq'|'pk'|'pv'|'z', i) -> psum tile
        t_azl = {}   # i -> azl (a+l psum)
        t_rcp = {}   # i -> rcp
        t_an = {}    # i -> an
        t_anT = {}   # i -> anT
        t_am = {}    # i -> am
        t_zo = {}    # g -> zo

        def emit_load(i):
            xc = xin.tile([128, 3, 3, D], F32, tag="xc")
            nc.sync.dma_start(
                xc[:], qkv_in[i].rearrange("t (c o) w d -> (o w) t c d", o=2))
            t_xin[i] = xc

        def emit_stats(i):
            g, j = divmod(i, GRP)
            if j == 0:
                st = stp.tile([128, GRP * 9, 6], F32, tag="st")
                mv = stp.tile([128, GRP * 9, 2], F32, tag="mv")
                lnv = stp.tile([128, GRP * 9], F32, tag="lnv")
                r9 = stp.tile([128, GRP * 9], F32, tag="r9")
                t_st[g] = (st, mv, lnv, r9)
            st, mv, lnv, r9 = t_st[g]
            xc = t_xin[i]
            for ti in range(3):
                for c in range(3):
                    s = j * 9 + ti * 3 + c
                    nc.vector.bn_stats(st[:, s, :], xc[:, ti, c, :])
                    nc.vector.bn_aggr(mv[:, s, :], st[:, s, :])

        def emit_rstd(g):
            st, mv, lnv, r9 = t_st[g]
            # rstd = exp(-0.5*ln(var+eps)); Ln/Exp share one ACT table set
            nc.scalar.activation(lnv[:], mv[:, :, 1], AF.Ln, bias=eps_sb[:, 0:1])
            nc.scalar.activation(r9[:], lnv[:], AF.Exp, scale=-0.5)

        def emit_norm(i):
            g, j = divmod(i, GRP)
            st, mv, lnv, r9 = t_st[g]
            xn = xnp.tile([128, 9, D], F16, tag="xn")
            xc = t_xin.pop(i)
            for ti in range(3):
                for c in range(3):
                    s = ti * 3 + c
                    sj = j * 9 + s
                    nc.gpsimd.tensor_scalar(
                        xn[:, s, :], xc[:, ti, c, :],
                        mv[:, sj, 0:1], r9[:, sj:sj + 1],
                        op0=ALU.subtract, op1=ALU.mult)
            t_xn[i] = xn

        def emit_xT(i):
            xT = xtp.tile([D, 9, 128], F16, tag="xT")
            nc.sync.dma_start_transpose(
                xT[:], t_xn[i][:].rearrange("p s d -> p (s d)"))
            t_xt[i] = xT
            t_xn.pop(i, None)

        def emit_proj_q(i):
            pq = p_ps.tile([128, Q], F32, tag="P")
            nc.tensor.matmul(pq[:], wq_sb[:],
                             t_xt[i][:, 0:3, :].rearrange("d c p -> d (c p)"),
                             start=True, stop=True)
            t_P[("pq", i)] = pq

        def emit_proj_kv(i):
            pk = p_ps.tile([128, Q], F32, tag="P")
            nc.tensor.matmul(pk[:], wk_sb[:],
                             t_xt[i][:, 3:6, :].rearrange("d c p -> d (c p)"),
                             start=True, stop=True)
            t_P[("pk", i)] = pk
            pv = p_ps.tile([128, Q], F32, tag="P")
            pv3 = pv[:].rearrange("p (c f) -> p c f", c=3)
            for c in range(3):
                nc.tensor.matmul(pv3[:, c, :], t_xt[i][:, 6 + c, :], wv_sb[:],
                                 start=True, stop=True)
            t_P[("pv", i)] = pv

        def emit_copies(i):
            qT = qkp.tile([H * DH, Q], F16, tag="qT")
            kT = qkp.tile([H * DH, Q], F16, tag="kT")
            nc.gpsimd.tensor_copy(qT[:], t_P.pop(("pq", i))[:])
            nc.gpsimd.tensor_copy(kT[:], t_P.pop(("pk", i))[:])
            vfa = vfp.tile([128, 3, H, DH + 1], F16, tag="vfa")
            pv3 = t_P.pop(("pv", i))[:].rearrange("p (c h x) -> p c h x", c=3, h=H)
            nc.gpsimd.tensor_copy(vfa[:, :, :, 0:DH], pv3[:])
            t_qk[i] = (qT, kT)
            t_vfa[i] = vfa
            t_xt.pop(i, None)

        def emit_scores_unit(i, u):
            """One (kc, head-pair) scores unit: 2 matmuls into a 2-bank tile."""
            kc, hp = divmod(u, 2)
            qT, kT = t_qk[i]
            sc = sc_ps.tile([128, 2, 512], F32, tag="sc")
            for hm in range(2):
                h = 2 * hp + hm
                nc.tensor.matmul(
                    sc[:, hm, 0:Q],
                    kT[32 * h:32 * (h + 1), kc * 128:(kc + 1) * 128],
                    qT[32 * h:32 * (h + 1), :],
                    start=True, stop=True, tile_position=(32 * h, 0))
            return sc

        def emit_exp_unit(i, u, sc):
            kc, hp = divmod(u, 2)
            if u == 0:
                t_E[i] = epool.tile([128, 3, 2, 2, Q], F16, tag="E", name="E")
            nc.scalar.activation(t_E[i][:, kc, hp], sc[:, :, 0:Q], AF.Exp)

        def emit_A(i):
            azl = azl_ps.tile([128, 3, H, DH + 1], F32, tag="azl")
            E = t_E[i]
            vfa = t_vfa[i]
            for qc in range(3):
                for h in range(H):
                    hp, hm = divmod(h, 2)
                    for kc in range(3):
                        nc.tensor.matmul(
                            azl[:, qc, h, :],
                            E[:, kc, hp, hm, qc * 128:(qc + 1) * 128],
                            vfa[:, kc, h, :],
                            start=(kc == 0), stop=(kc == 2))
            t_azl[i] = azl

        def emit_rcp_mult(i):
            azl = t_azl.pop(i)
            rcp = rcpp.tile([128, 3, H], F32, tag="rcp")
            nc.vector.reciprocal_approx_fast(out=rcp[:], in_=azl[:, :, :, DH])
            an = anp.tile([128, 3, H, DH], F16, tag="an")
            for qc in range(3):
                nc.vector.tensor_tensor(
                    an[:, qc], azl[:, qc, :, 0:DH],
                    rcp[:, qc, :].unsqueeze(-1).broadcast_to([128, H, DH]),
                    op=ALU.mult)
            t_an[i] = an
            t_E.pop(i, None)
            t_vfa.pop(i, None)

        def emit_anT(i):
            anT = antp.tile([H * DH, 3, 128], F16, tag="anT")
            nc.sync.dma_start_transpose(
                anT[:], t_an[i][:].rearrange("p c h x -> p (c h x)"))
            t_anT[i] = anT
            t_an.pop(i, None)

        def emit_amred(i):
            am = amp.tile([H * DH, W], F16, tag="am")
            with nc.allow_low_precision(reason="6-term fp16 mean, 2e-2 tol"):
                nc.vector.tensor_reduce(
                    am[:], t_anT[i][:].rearrange("p c (o w) -> p w (c o)", o=2),
                    axis=AXX, op=ALU.add)
            t_am[i] = am
            t_anT.pop(i, None)

        def emit_z(i):
            z = p_ps.tile([128, Q], F32, tag="P")
            nc.tensor.matmul(z[:, 0:W], wp_sb[:], t_am[i][:],
                             start=True, stop=True)
            t_P[("z", i)] = z
            t_am.pop(i, None)

        def emit_zo(i):
            g, j = divmod(i, GRP)
            if j == 0:
                t_zo[g] = zop.tile([D, GRP, W], F32, tag="zo", name="zo")
            nc.vector.tensor_copy(t_zo[g][:, j, :], t_P.pop(("z", i))[:, 0:W])

        def emit_out(g):
            gs = slice(g * GRP, (g + 1) * GRP)
            nc.sync.dma_start(out_d[gs].rearrange("g d w -> d g w"),
                              t_zo.pop(g)[:])

        n = n_cells
        t_sc = {}    # i -> [6 sc tiles], produced one tick early
        for t in range(-9, n + 3):
            # -- ACT: grouped rstd FIRST (stats landed 2 ticks ago), then exps --
            if t % 4 == 0:
                g_ln = t // 4 + 1
                if 0 <= g_ln < n_grp:
                    emit_rstd(g_ln)
            if 0 <= t < n:
                for u in range(6):
                    emit_exp_unit(t, u, t_sc[t][u])
                del t_sc[t]
            # -- DVE head: amred(t-2), rcp/mult(t-1); SP head: anT(t-1) --
            if 0 <= t - 2 < n:
                emit_amred(t - 2)
            if 0 <= t - 1 < n:
                emit_A(t - 1)
                emit_rcp_mult(t - 1)
                emit_anT(t - 1)
            # -- SP: load(t+9), group out at t = 4g+6 --
            if 0 <= t + 9 < n:
                emit_load(t + 9)
            if t % 4 == 2:
                g_out = (t - 6) // 4
                if 0 <= g_out < n_grp:
                    emit_out(g_out)
            # -- PE: proj(t+2) (P-ring acq order: pq,pk,pv, z last) --
            if 0 <= t + 2 < n:
                emit_proj_q(t + 2)
                emit_proj_kv(t + 2)
            # -- Pool: norm(t+3) then copies(t+2) --
            if 0 <= t + 3 < n:
                emit_norm(t + 3)
            if 0 <= t + 2 < n:
                emit_copies(t + 2)
            # -- SP: xT(t+3) --
            if 0 <= t + 3 < n:
                emit_xT(t + 3)
            # -- PE tail: scores(t+1), JIT-paced by the sc ring vs exps(t) --
            if 0 <= t + 1 < n:
                t_sc[t + 1] = [emit_scores_unit(t + 1, u) for u in range(6)]
            # -- DVE: stats(t+9); PE tail: z(t-2); DVE tail: zo(t-2) --
            if 0 <= t + 9 < n:
                emit_stats(t + 9)
            if 0 <= t - 2 < n:
                emit_z(t - 2)
                emit_zo(t - 2)

    nc.compile()
    return nc


_NC_CACHE = {}


def _get_nc(n_cells: int):
    if n_cells not in _NC_CACHE:
        _NC_CACHE[n_cells] = _build(n_cells)
    return _NC_CACHE[n_cells]


def _fold_weights(head_gate, lnq_g, lnq_b, lnk_g, lnk_b, lnv_g, lnv_b,
                  Wq, bq, Wk, bk, Wv, bv, Wp, bp):
    """Fold LN affine, head gate, scale, and 1/6-mean into the weights."""
    scale = DH ** -0.5
    gh = np.repeat(np.asarray(head_gate, np.float64), DH)        # [H*DH]
    sq = np.sqrt(scale)

    def proj(g, b, Wx, bx, colscale):
        Wf = (np.asarray(g, np.float64)[:, None] * np.asarray(Wx, np.float64)) * colscale
        bf = (np.asarray(b, np.float64) @ np.asarray(Wx, np.float64)
              + np.asarray(bx, np.float64)) * colscale
        return Wf, bf

    Wq2, bq2 = proj(lnq_g, lnq_b, Wq, bq, gh * sq)
    Wk2, bk2 = proj(lnk_g, lnk_b, Wk, bk, gh * sq)
    Wv2, bv2 = proj(lnv_g, lnv_b, Wv, bv, gh)
    Wp2 = np.asarray(Wp, np.float64) / 6.0
    bp2 = np.asarray(bp, np.float64) + bv2 @ np.asarray(Wp, np.float64)
    assert np.abs(bq2).max() < 1e-7 and np.abs(bk2).max() < 1e-7, \
        "nonzero q/k biases not supported by this kernel build"
    return (Wq2.astype(np.float16), Wk2.astype(np.float16),
            Wv2.astype(np.float16),
            Wp2.astype(np.float16), bp2.astype(np.float32))


def make_in_maps(q, k, v, skip, head_gate,
                 lnq_g, lnq_b, lnk_g, lnk_b, lnv_g, lnv_b,
                 Wq, bq, Wk, bk, Wv, bv, Wp, bp):
    q = np.asarray(q); k = np.asarray(k); v = np.asarray(v)
    Wq2, Wk2, Wv2, Wp2, bp2 = _fold_weights(
        head_gate, lnq_g, lnq_b, lnk_g, lnk_b, lnv_g, lnv_b,
        Wq, bq, Wk, bk, Wv, bv, Wp, bp)

    # cells: global index g = b*64 + x*8 + y -> core g//16, slot g%16
    qkv = np.stack([np.asarray(q), np.asarray(k), np.asarray(v)], axis=2)
    # [B, N, 3, L, W, D] -> [B*L, 3, N, W, D]
    qkv = qkv.reshape(B, N, 3, L, W, D).transpose(0, 3, 2, 1, 4, 5)
    qkv = qkv.reshape(B * L, 3, N, W, D)

    consts = {
        "wq": Wq2, "wk": Wk2, "wv": Wv2, "wp": Wp2,
        "epsc": np.full((128, 1), EPS, np.float32),
    }
    in_maps = []
    for r in range(NCORES):
        s = slice(r * CELLS, (r + 1) * CELLS)
        in_maps.append({
            "qkv_in": np.ascontiguousarray(qkv[s], np.float32),
            **consts,
        })
    return in_maps, bp2


def kernel(**inputs):
    in_maps, bp2 = make_in_maps(**inputs)
    skip = np.asarray(inputs["skip"])
    nc = _get_nc(CELLS)
    res = run_bass_kernel_spmd(nc, in_maps, core_ids=list(range(NCORES)))
    outs = np.stack([res.results[r]["out"] for r in range(NCORES)])  # [8,16,D,W]
    z = outs.reshape(B * L, D, W).transpose(0, 2, 1)                 # [128,W,D]
    z = z + bp2[None, None, :].astype(np.float32)
    z = z.reshape(B, X, Y, W1, W2, D).astype(np.float32) + skip
    return z


# revision 22
# speedup vs baseline: 1.1094x; 1.1094x over previous
"""Trainium2 Bass kernel for nn_CrossWinAttention (windowed cross attention).

Contract: kernel(**inputs) takes FULL numpy inputs (as produced by
setup_inputs()) and returns the FULL output of the reference nn.Module.

Sharding: the (b, x*y) = 2*64 = 128 window cells are fully independent
(LN, QKV proj, per-cell attention, proj, mean over n are all cell-local).
16 cells per core on 8 NeuronCores; no collectives.

v2 design (software-pipelined, ACT-bound):
  Per cell i (tokens t = 64n + 8*w1 + w2, chunks c of 128):
    x      [128p, 3c, 128d]  f32 token-major (p = token % 128)
    LN stats on DVE (bn_stats/aggr, grouped rstd on ACT)
    normalize on Pool -> xn f16, one blocked DMA-transpose -> xT
    qT/kT = W^T xT (PE, psum ring) -> fp16 SBUF copies on Pool
    vf = xT_v chunks @ Wv -> [tok,hd] directly (no transpose), Pool copy
       into vf_aug [128,3,4,33] whose 33rd column is preset to 1.0
    scores^T per (kc, head-pair) into [128,2,512] PSUM (2-bank tiles,
       double buffered), exp on ACT -> E fp16 (6 exps/cell, the wall)
    A+l: out[q,33] = E_chunk^T @ [vf_h|1] accumulated over kc (36 matmuls
       of 33 cols -- l lands as column 32)
    rcp l on DVE (reciprocal_approx_fast), an = a * rcp (broadcast AP)
    an -> anT (DMA transpose), mean over n via strided DVE reduce -> am,
    z^T = Wp^T am (64 cols), DVE copy to zo, group DMA out.
  Engine streams are explicitly modulo-scheduled across cells so ACT
  (exp) never starves; PSUM is exactly 8 banks.
"""

import sys
import contextlib
import functools

sys.path.insert(0, "/opt/trn_rl_repo")

import numpy as np

import concourse.hw_specs as hw_specs

# --- ACT table-set pin ------------------------------------------------------
# bacc's insert_act_table_loads greedily picks the first table set containing
# each activation function; Ln and Exp then alternate between two sets and
# every cell pays ~2.7us table reloads.  Restrict Exp/Ln membership to the
# combined set (its index — and therefore the emitted act_func_set_id — is
# unchanged) so one load serves the whole kernel.
_orig_get_tables = hw_specs.get_activation_tables


@functools.cache
def _pinned_tables(arch):
    import concourse.mybir as mybir
    AF = mybir.ActivationFunctionType
    tabs = dict(_orig_get_tables(arch))
    out = {}
    for name, fns in tabs.items():
        if name != "natural_log_exp_and_others":
            fns = fns - {AF.Exp, AF.Ln}
        out[name] = fns
    return out


hw_specs.get_activation_tables = _pinned_tables

import concourse.bass as bass
import concourse.tile as tile
from concourse import bacc, mybir
from concourse.bass_utils import run_bass_kernel_spmd

# Problem shape (hardcoded per spec nn_CrossWinAttention_81346680586407)
B, N, X, Y, W1, W2, D = 2, 6, 8, 8, 8, 8, 128
H, DH = 4, 32
EPS = 1e-5
L = X * Y              # 64 cells per batch
Q = N * W1 * W2        # 384 tokens per cell
W = W1 * W2            # 64 output positions per cell
NCORES = 8
CELLS = (B * L) // NCORES   # 16 cells per core
GRP = 4                     # cells per input-DMA batch
F32 = mybir.dt.float32
F16 = mybir.dt.float16
AF = mybir.ActivationFunctionType
ALU = mybir.AluOpType
AXX = mybir.AxisListType.X


def _build(n_cells: int):
    """Build the per-core Bass graph (same program on all 8 cores)."""
    nc = bacc.Bacc("TRN2", target_bir_lowering=False, debug=False,
                   num_devices=NCORES)

    qkv_in = nc.dram_tensor("qkv_in", [n_cells, 3, N, W, D], F32,
                            kind="ExternalInput").ap()
    wq_d = nc.dram_tensor("wq", [D, H * DH], F16, kind="ExternalInput").ap()
    wk_d = nc.dram_tensor("wk", [D, H * DH], F16, kind="ExternalInput").ap()
    wv_d = nc.dram_tensor("wv", [D, H * DH], F16, kind="ExternalInput").ap()
    wp_d = nc.dram_tensor("wp", [H * DH, D], F16, kind="ExternalInput").ap()
    id_d = nc.dram_tensor("ident", [128, 128], F16, kind="ExternalInput").ap()
    eps_d = nc.dram_tensor("epsc", [128, 1], F32, kind="ExternalInput").ap()
    out_d = nc.dram_tensor("out", [n_cells, D, W], F32, kind="ExternalOutput").ap()

    n_grp = (n_cells + GRP - 1) // GRP

    with tile.TileContext(nc) as tc, contextlib.ExitStack() as ctx:
        # ---- PSUM: exactly 8 banks -----------------------------------------
        # sc: 2-bank tiles x2 bufs = 4; P-ring [128,384] x3 = 3; azl 1.
        sc_ps = ctx.enter_context(tc.tile_pool(name="sc_ps", bufs=2, space="PSUM"))
        p_ps = ctx.enter_context(tc.tile_pool(name="p_ps", bufs=3, space="PSUM"))
        azl_ps = ctx.enter_context(tc.tile_pool(name="azl_ps", bufs=1, space="PSUM"))

        # ---- SBUF pools ----------------------------------------------------
        cpool = ctx.enter_context(tc.tile_pool(name="consts", bufs=1))
        xin = ctx.enter_context(tc.tile_pool(name="xin", bufs=8))
        stp = ctx.enter_context(tc.tile_pool(name="st", bufs=3))
        xnp = ctx.enter_context(tc.tile_pool(name="xn", bufs=3))
        xtp = ctx.enter_context(tc.tile_pool(name="xt", bufs=3))
        qkp = ctx.enter_context(tc.tile_pool(name="qk", bufs=3))
        vfp = ctx.enter_context(tc.tile_pool(name="vf", bufs=4))
        epool = ctx.enter_context(tc.tile_pool(name="E", bufs=3))
        rcpp = ctx.enter_context(tc.tile_pool(name="rcp", bufs=2))
        anp = ctx.enter_context(tc.tile_pool(name="an", bufs=2))
        amp = ctx.enter_context(tc.tile_pool(name="am", bufs=2))
        zop = ctx.enter_context(tc.tile_pool(name="zo", bufs=2))

        # ---- constants -----------------------------------------------------
        wq_sb = cpool.tile([D, H * DH], F16)
        wk_sb = cpool.tile([D, H * DH], F16)
        wv_sb = cpool.tile([D, H * DH], F16)
        wp_sb = cpool.tile([H * DH, D], F16)
        id_sb = cpool.tile([128, 128], F16)
        eps_sb = cpool.tile([128, 1], F32)
        for sb, dr in ((wq_sb, wq_d), (wk_sb, wk_d), (wv_sb, wv_d),
                       (wp_sb, wp_d), (id_sb, id_d), (eps_sb, eps_d)):
            nc.sync.dma_start(sb[:], dr[:])

        # vf_aug ring: preset the ones column once per buffer (never
        # overwritten afterwards -- the per-cell copy writes only [:, :, :, 0:32]).
        VF_BUFS = 4
        vfa_ring = []
        for _ in range(VF_BUFS):
            vfa0 = vfp.tile([128, 3, H, DH + 1], F16, tag="vfa")
            nc.gpsimd.memset(vfa0[:, :, :, DH:DH + 1], 1.0)
            vfa_ring.append(vfa0)

        # per-cell / per-group tile handles
        t_xin = {}   # i -> xin [128, 3t, 3c, D]
        t_st = {}    # g -> (st, mv, lnv, r9)
        t_xn = {}    # i -> xn
        t_xt = {}    # i -> xT
        t_qk = {}    # i -> (qT, kT)
        t_vfa = {}   # i -> vf_aug
        t_E = {}     # i -> E
        t_P = {}     # ('pq'|'pk'|'pv'|'z', i) -> psum tile
        t_azl = {}   # i -> azl (a+l psum)
        t_rcp = {}   # i -> rcp
        t_an = {}    # i -> an
        t_anT = {}   # i -> anT
        t_am = {}    # i -> am
        t_zo = {}    # g -> zo

        def emit_load(i):
            xc = xin.tile([128, 3, 3, D], F32, tag="xc")
            nc.sync.dma_start(
                xc[:], qkv_in[i].rearrange("t (c o) w d -> (o w) t c d", o=2))
            t_xin[i] = xc

        def emit_stats(i):
            g, j = divmod(i, GRP)
            if j == 0:
                st = stp.tile([128, GRP * 9, 6], F32, tag="st")
                mv = stp.tile([128, GRP * 9, 2], F32, tag="mv")
                lnv = stp.tile([128, GRP * 9], F32, tag="lnv")
                r9 = stp.tile([128, GRP * 9], F32, tag="r9")
                t_st[g] = (st, mv, lnv, r9)
            st, mv, lnv, r9 = t_st[g]
            xc = t_xin[i]
            for ti in range(3):
                for c in range(3):
                    s = j * 9 + ti * 3 + c
                    nc.vector.bn_stats(st[:, s, :], xc[:, ti, c, :])
                    nc.vector.bn_aggr(mv[:, s, :], st[:, s, :])

        def emit_rstd(g):
            st, mv, lnv, r9 = t_st[g]
            # rstd = exp(-0.5*ln(var+eps)); Ln/Exp share one ACT table set
            nc.scalar.activation(lnv[:], mv[:, :, 1], AF.Ln, bias=eps_sb[:, 0:1])
            nc.scalar.activation(r9[:], lnv[:], AF.Exp, scale=-0.5)

        def emit_norm(i):
            g, j = divmod(i, GRP)
            st, mv, lnv, r9 = t_st[g]
            xn = xnp.tile([128, 9, D], F16, tag="xn")
            xc = t_xin.pop(i)
            for ti in range(3):
                for c in range(3):
                    s = ti * 3 + c
                    sj = j * 9 + s
                    nc.gpsimd.tensor_scalar(
                        xn[:, s, :], xc[:, ti, c, :],
                        mv[:, sj, 0:1], r9[:, sj:sj + 1],
                        op0=ALU.subtract, op1=ALU.mult)
            t_xn[i] = xn

        def emit_xT(i):
            xT = xtp.tile([D, 9, 128], F16, tag="xT")
            nc.sync.dma_start_transpose(
                xT[:], t_xn[i][:].rearrange("p s d -> p (s d)"))
            t_xt[i] = xT
            t_xn.pop(i, None)

        def emit_proj_q(i):
            pq = p_ps.tile([128, Q], F32, tag="P")
            nc.tensor.matmul(pq[:], wq_sb[:],
                             t_xt[i][:, 0:3, :].rearrange("d c p -> d (c p)"),
                             start=True, stop=True)
            t_P[("pq", i)] = pq

        def emit_proj_kv(i):
            pk = p_ps.tile([128, Q], F32, tag="P")
            nc.tensor.matmul(pk[:], wk_sb[:],
                             t_xt[i][:, 3:6, :].rearrange("d c p -> d (c p)"),
                             start=True, stop=True)
            t_P[("pk", i)] = pk
            pv = p_ps.tile([128, Q], F32, tag="P")
            pv3 = pv[:].rearrange("p (c f) -> p c f", c=3)
            for c in range(3):
                nc.tensor.matmul(pv3[:, c, :], t_xt[i][:, 6 + c, :], wv_sb[:],
                                 start=True, stop=True)
            t_P[("pv", i)] = pv

        def emit_copies(i):
            qT = qkp.tile([H * DH, Q], F16, tag="qT")
            kT = qkp.tile([H * DH, Q], F16, tag="kT")
            nc.gpsimd.tensor_copy(qT[:], t_P.pop(("pq", i))[:])
            nc.gpsimd.tensor_copy(kT[:], t_P.pop(("pk", i))[:])
            vfa = vfp.tile([128, 3, H, DH + 1], F16, tag="vfa")
            pv3 = t_P.pop(("pv", i))[:].rearrange("p (c h x) -> p c h x", c=3, h=H)
            nc.gpsimd.tensor_copy(vfa[:, :, :, 0:DH], pv3[:])
            t_qk[i] = (qT, kT)
            t_vfa[i] = vfa
            t_xt.pop(i, None)

        def emit_scores_unit(i, u):
            """One (kc, head-pair) scores unit: 2 matmuls into a 2-bank tile."""
            kc, hp = divmod(u, 2)
            qT, kT = t_qk[i]
            sc = sc_ps.tile([128, 2, 512], F32, tag="sc")
            for hm in range(2):
                h = 2 * hp + hm
                nc.tensor.matmul(
                    sc[:, hm, 0:Q],
                    kT[32 * h:32 * (h + 1), kc * 128:(kc + 1) * 128],
                    qT[32 * h:32 * (h + 1), :],
                    start=True, stop=True, tile_position=(32 * h, 0))
            return sc

        def emit_exp_unit(i, u, sc):
            kc, hp = divmod(u, 2)
            if u == 0:
                t_E[i] = epool.tile([128, 3, 2, 2, Q], F16, tag="E", name="E")
            nc.scalar.activation(t_E[i][:, kc, hp], sc[:, :, 0:Q], AF.Exp)

        def emit_A(i):
            azl = azl_ps.tile([128, 3, H, DH + 1], F32, tag="azl")
            E = t_E[i]
            vfa = t_vfa[i]
            for qc in range(3):
                for h in range(H):
                    hp, hm = divmod(h, 2)
                    for kc in range(3):
                        nc.tensor.matmul(
                            azl[:, qc, h, :],
                            E[:, kc, hp, hm, qc * 128:(qc + 1) * 128],
                            vfa[:, kc, h, :],
                            start=(kc == 0), stop=(kc == 2))
            t_azl[i] = azl

        def emit_rcp_mult(i):
            azl = t_azl.pop(i)
            rcp = rcpp.tile([128, 3, H], F32, tag="rcp")
            nc.vector.reciprocal_approx_fast(out=rcp[:], in_=azl[:, :, :, DH])
            an = anp.tile([128, 3, H, DH], F16, tag="an")
            for qc in range(3):
                nc.vector.tensor_tensor(
                    an[:, qc], azl[:, qc, :, 0:DH],
                    rcp[:, qc, :].unsqueeze(-1).broadcast_to([128, H, DH]),
                    op=ALU.mult)
            t_an[i] = an
            t_E.pop(i, None)
            t_vfa.pop(i, None)

        def emit_anT(i):
            anT = p_ps.tile([H * DH, 3, 128], F16, tag="P", name="anT")
            an = t_an[i]
            for qc in range(3):
                nc.tensor.transpose(anT[:, qc, :],
                                    an[:, qc].rearrange("p h x -> p (h x)"),
                                    id_sb[:])
            t_anT[i] = anT
            t_an.pop(i, None)

        def emit_amred(i):
            am = amp.tile([H * DH, W], F16, tag="am")
            with nc.allow_low_precision(reason="6-term fp16 mean, 2e-2 tol"):
                nc.vector.tensor_reduce(
                    am[:], t_anT[i][:].rearrange("p c (o w) -> p w (c o)", o=2),
                    axis=AXX, op=ALU.add)
            t_am[i] = am
            t_anT.pop(i, None)

        def emit_z(i):
            z = p_ps.tile([128, Q], F32, tag="P")
            nc.tensor.matmul(z[:, 0:W], wp_sb[:], t_am[i][:],
                             start=True, stop=True)
            t_P[("z", i)] = z
            t_am.pop(i, None)

        def emit_zo(i):
            g, j = divmod(i, GRP)
            if j == 0:
                t_zo[g] = zop.tile([D, GRP, W], F32, tag="zo", name="zo")
            nc.vector.tensor_copy(t_zo[g][:, j, :], t_P.pop(("z", i))[:, 0:W])

        def emit_out(g):
            gs = slice(g * GRP, (g + 1) * GRP)
            nc.sync.dma_start(out_d[gs].rearrange("g d w -> d g w"),
                              t_zo.pop(g)[:])

        n = n_cells
        t_sc = {}    # i -> [6 sc tiles], produced one tick early
        for t in range(-9, n + 3):
            # -- ACT: grouped rstd FIRST (stats landed 2 ticks ago), then exps --
            if t % 4 == 0:
                g_ln = t // 4 + 1
                if 0 <= g_ln < n_grp:
                    emit_rstd(g_ln)
            if 0 <= t < n:
                for u in range(6):
                    emit_exp_unit(t, u, t_sc[t][u])
                del t_sc[t]
            # -- DVE head: amred(t-2), rcp/mult(t-1); SP head: anT(t-1) --
            if 0 <= t - 2 < n:
                emit_amred(t - 2)
            if 0 <= t - 1 < n:
                emit_A(t - 1)
                emit_rcp_mult(t - 1)
                emit_anT(t - 1)
            # -- SP: load(t+9), group out at t = 4g+6 --
            if 0 <= t + 9 < n:
                emit_load(t + 9)
            if t % 4 == 2:
                g_out = (t - 6) // 4
                if 0 <= g_out < n_grp:
                    emit_out(g_out)
            # -- PE: proj(t+2) (P-ring acq order: pq,pk,pv, z last) --
            if 0 <= t + 2 < n:
                emit_proj_q(t + 2)
                emit_proj_kv(t + 2)
            # -- Pool: norm(t+3) then copies(t+2) --
            if 0 <= t + 3 < n:
                emit_norm(t + 3)
            if 0 <= t + 2 < n:
                emit_copies(t + 2)
            # -- SP: xT(t+3) --
            if 0 <= t + 3 < n:
                emit_xT(t + 3)
            # -- PE tail: scores(t+1), JIT-paced by the sc ring vs exps(t) --
            if 0 <= t + 1 < n:
                t_sc[t + 1] = [emit_scores_unit(t + 1, u) for u in range(6)]
            # -- DVE: stats(t+9); PE tail: z(t-2); DVE tail: zo(t-2) --
            if 0 <= t + 9 < n:
                emit_stats(t + 9)
            if 0 <= t - 2 < n:
                emit_z(t - 2)
                emit_zo(t - 2)

    nc.compile()
    return nc


_NC_CACHE = {}


def _get_nc(n_cells: int):
    if n_cells not in _NC_CACHE:
        _NC_CACHE[n_cells] = _build(n_cells)
    return _NC_CACHE[n_cells]


def _fold_weights(head_gate, lnq_g, lnq_b, lnk_g, lnk_b, lnv_g, lnv_b,
                  Wq, bq, Wk, bk, Wv, bv, Wp, bp):
    """Fold LN affine, head gate, scale, and 1/6-mean into the weights."""
    scale = DH ** -0.5
    gh = np.repeat(np.asarray(head_gate, np.float64), DH)        # [H*DH]
    sq = np.sqrt(scale)

    def proj(g, b, Wx, bx, colscale):
        Wf = (np.asarray(g, np.float64)[:, None] * np.asarray(Wx, np.float64)) * colscale
        bf = (np.asarray(b, np.float64) @ np.asarray(Wx, np.float64)
              + np.asarray(bx, np.float64)) * colscale
        return Wf, bf

    Wq2, bq2 = proj(lnq_g, lnq_b, Wq, bq, gh * sq)
    Wk2, bk2 = proj(lnk_g, lnk_b, Wk, bk, gh * sq)
    Wv2, bv2 = proj(lnv_g, lnv_b, Wv, bv, gh)
    Wp2 = np.asarray(Wp, np.float64) / 6.0
    bp2 = np.asarray(bp, np.float64) + bv2 @ np.asarray(Wp, np.float64)
    assert np.abs(bq2).max() < 1e-7 and np.abs(bk2).max() < 1e-7, \
        "nonzero q/k biases not supported by this kernel build"
    return (Wq2.astype(np.float16), Wk2.astype(np.float16),
            Wv2.astype(np.float16),
            Wp2.astype(np.float16), bp2.astype(np.float32))


def make_in_maps(q, k, v, skip, head_gate,
                 lnq_g, lnq_b, lnk_g, lnk_b, lnv_g, lnv_b,
                 Wq, bq, Wk, bk, Wv, bv, Wp, bp):
    q = np.asarray(q); k = np.asarray(k); v = np.asarray(v)
    Wq2, Wk2, Wv2, Wp2, bp2 = _fold_weights(
        head_gate, lnq_g, lnq_b, lnk_g, lnk_b, lnv_g, lnv_b,
        Wq, bq, Wk, bk, Wv, bv, Wp, bp)

    # cells: global index g = b*64 + x*8 + y -> core g//16, slot g%16
    qkv = np.stack([np.asarray(q), np.asarray(k), np.asarray(v)], axis=2)
    # [B, N, 3, L, W, D] -> [B*L, 3, N, W, D]
    qkv = qkv.reshape(B, N, 3, L, W, D).transpose(0, 3, 2, 1, 4, 5)
    qkv = qkv.reshape(B * L, 3, N, W, D)

    consts = {
        "wq": Wq2, "wk": Wk2, "wv": Wv2, "wp": Wp2,
        "ident": np.eye(128, dtype=np.float16),
        "epsc": np.full((128, 1), EPS, np.float32),
    }
    in_maps = []
    for r in range(NCORES):
        s = slice(r * CELLS, (r + 1) * CELLS)
        in_maps.append({
            "qkv_in": np.ascontiguousarray(qkv[s], np.float32),
            **consts,
        })
    return in_maps, bp2


def kernel(**inputs):
    in_maps, bp2 = make_in_maps(**inputs)
    skip = np.asarray(inputs["skip"])
    nc = _get_nc(CELLS)
    res = run_bass_kernel_spmd(nc, in_maps, core_ids=list(range(NCORES)))
    outs = np.stack([res.results[r]["out"] for r in range(NCORES)])  # [8,16,D,W]
    z = outs.reshape(B * L, D, W).transpose(0, 2, 1)                 # [128,W,D]
    z = z + bp2[None, None, :].astype(np.float32)
    z = z.reshape(B, X, Y, W1, W2, D).astype(np.float32) + skip
    return z
